# revision 1
# baseline (speedup 1.0000x reference)
"""BatchSRU Trainium2 kernel (nn_BatchSRU_27556510171508).

Full inputs: x (2048, 8, 128, 16) f32, W (16, 128, 384), b (16, 256).
Sharding: data-parallel over the inner batch B=8 -> one batch row per
NeuronCore (zero cross-core communication); W/b replicated.

Per-core dataflow (L=2048, D=128, NB=16):
  - x slice loaded contiguously as [l-part, (d nb)] tiles
  - PE transposes to channel layout [d-part, l-free] per instance
  - ACT copies PSUM->SBUF rounding to f32r (matmul rhs)
  - f32r matmuls per (instance, gate): U^T = W^T @ x^T
  - ACT sigmoids with fused per-partition bias
  - one wide tensor_tensor_scan per 4-instance group covers all scan
    channels (zero-injection at instance boundaries + carry fixup)
  - highway h = x + r*(c-x); the final add reads the PE-transposed
    r*(c-x) from PSUM and accumulates onto the resident x tile, which
    is then stored contiguously
"""

import numpy as np
from contextlib import ExitStack

import concourse.bacc as bacc
import concourse.tile as tile
from concourse import mybir
from concourse.masks import make_identity

F32 = mybir.dt.float32
F32R = mybir.dt.float32r
AL = mybir.AluOpType
AF = mybir.ActivationFunctionType

L, B, D, NB = 2048, 8, 128, 16
LC = 512                 # l-chunk
NCH = L // LC            # 4 chunks
QNB = 4                  # instances per group
NQ = NB // QNB           # 4 groups
NLS = LC // 128          # 4 l-subtiles per chunk

N_CORES = 8


def _build(repeat: int = 1):
    nc = bacc.Bacc("TRN2")
    x = nc.dram_tensor("x", [L, D, NB], F32, kind="ExternalInput")
    w = nc.dram_tensor("w", [NB, D, 3 * D], F32, kind="ExternalInput")
    bb = nc.dram_tensor("bb", [NB, 2 * D], F32, kind="ExternalInput")
    out = nc.dram_tensor("out", [L, D, NB], F32, kind="ExternalOutput")

    with tile.TileContext(nc) as tc, ExitStack() as ctx:
        const = ctx.enter_context(tc.tile_pool(name="const", bufs=1))

        ident = const.tile([128, 128], F32)
        make_identity(nc, ident)
        wr = const.tile([128, NB, 3 * D], F32R)
        bsb = const.tile([128, NB, 2], F32)
        nc.sync.dma_start(out=bsb, in_=bb.rearrange("n (g d) -> d n g", d=128))
        carry = const.tile([128, NB], F32)
        nc.vector.memset(carry, 0.0)

        with tc.tile_pool(name="wtmp_pool", bufs=1) as wtmp_pool:
            wtmp = wtmp_pool.tile([128, NB, 3 * D], F32)
            nc.sync.dma_start(out=wtmp, in_=w.transpose([1, 0, 2]))
            nc.vector.tensor_copy(wr, wtmp)

        xpool = ctx.enter_context(tc.tile_pool(name="xpool", bufs=2))
        sb = ctx.enter_context(tc.tile_pool(name="sb", bufs=2))
        pt = ctx.enter_context(tc.tile_pool(name="pt", bufs=1, space="PSUM"))
        pux = ctx.enter_context(tc.tile_pool(name="pux", bufs=2, space="PSUM"))
        pufr = ctx.enter_context(tc.tile_pool(name="pufr", bufs=2, space="PSUM"))
        ph = ctx.enter_context(tc.tile_pool(name="ph", bufs=1, space="PSUM"))

        import contextlib

        loop_cm = tc.For_i(0, repeat) if repeat > 1 else contextlib.nullcontext()
        with loop_cm:
         for lc in range(NCH):
            xts = []
            for ls in range(NLS):
                xt_in = xpool.tile([128, D * NB], F32, tag=f"X{ls}")
                l0 = lc * LC + ls * 128
                nc.sync.dma_start(
                    out=xt_in, in_=x[l0 : l0 + 128].rearrange("l d n -> l (d n)")
                )
                xts.append(xt_in)

            for q in range(NQ):
                xT = sb.tile([128, QNB, LC], F32R, tag="xT")
                fw = sb.tile([128, QNB, LC], F32, tag="f")
                gw = sb.tile([128, QNB, LC], F32, tag="g")
                rw = sb.tile([128, QNB, LC], F32, tag="r")
                cw = sb.tile([128, QNB, LC], F32, tag="c")

                for j in range(QNB):
                    nb = q * QNB + j
                    # in-transpose: 4 l-subtiles -> psum [d, LC]
                    pstage = pt.tile([128, LC], F32, tag="pt")
                    for ls in range(NLS):
                        xg = xts[ls].rearrange("l (d n) -> l n d", n=NB)
                        nc.tensor.transpose(
                            pstage[:, ls * 128 : (ls + 1) * 128], xg[:, nb], ident
                        )
                    # rounding copy psum -> f32r SBUF
                    nc.scalar.copy(xT[:, j], pstage)

                    # matmuls
                    ux = pux.tile([128, LC], F32, tag="ux")
                    ufr = pufr.tile([128, 2, LC], F32, tag="ufr")
                    nc.tensor.matmul(
                        ux, wr[:, nb, 0:128], xT[:, j], start=True, stop=True
                    )
                    nc.tensor.matmul(
                        ufr[:, 0], wr[:, nb, 128:256], xT[:, j], start=True, stop=True
                    )
                    nc.tensor.matmul(
                        ufr[:, 1], wr[:, nb, 256:384], xT[:, j], start=True, stop=True
                    )
                    # gates
                    nc.scalar.activation(
                        fw[:, j], ufr[:, 0], AF.Sigmoid, bias=bsb[:, nb, 0:1], scale=1.0
                    )
                    nc.scalar.activation(
                        rw[:, j], ufr[:, 1], AF.Sigmoid, bias=bsb[:, nb, 1:2], scale=1.0
                    )
                    # fbar = 1 - f (gpsimd), then g = fbar * x_tilde (VE, psum)
                    nc.gpsimd.tensor_scalar(
                        gw[:, j], fw[:, j], -1.0, 1.0, AL.mult, AL.add
                    )
                    nc.vector.tensor_tensor(gw[:, j], gw[:, j], ux, AL.mult)

                # scan fixup at l=0 columns of each instance in the group
                carry_q = carry[:, q * QNB : (q + 1) * QNB]
                fcols = fw.rearrange("p n l -> p l n")[:, 0]
                gcols = gw.rearrange("p n l -> p l n")[:, 0]
                tmp = sb.tile([128, QNB], F32, tag="tmp")
                nc.vector.tensor_tensor(tmp, fcols, carry_q, AL.mult)
                nc.vector.tensor_tensor(gcols, gcols, tmp, AL.add)
                nc.vector.memset(fcols, 0.0)
                # one wide scan for the whole group
                nc.vector.tensor_tensor_scan(
                    cw.rearrange("p n l -> p (n l)"),
                    fw.rearrange("p n l -> p (n l)"),
                    gw.rearrange("p n l -> p (n l)"),
                    0.0,
                    op0=AL.mult,
                    op1=AL.add,
                )
                nc.vector.tensor_copy(
                    carry_q, cw.rearrange("p n l -> p l n")[:, LC - 1]
                )

                # t = c - x (in place on c), u = r * t (in place on r)
                xTf = xT.bitcast(F32)
                nc.gpsimd.tensor_tensor(cw, cw, xTf, AL.subtract)
                nc.gpsimd.tensor_tensor(rw, rw, cw, AL.mult)

                # out-transpose u and fuse h = u^T + x onto the x tiles
                for ls in range(NLS):
                    hps = ph.tile([128, QNB * 128], F32, tag="ph")
                    for j in range(QNB):
                        nc.tensor.transpose(
                            hps[:, j * 128 : (j + 1) * 128],
                            rw[:, j, ls * 128 : ls * 128 + 128],
                            ident,
                        )
                    xv = xts[ls].rearrange("l (d n) -> l d n", n=NB)[
                        :, :, q * QNB : (q + 1) * QNB
                    ]
                    hv = hps.rearrange("l (n d) -> l d n", n=QNB)
                    nc.vector.tensor_tensor(xv, hv, xv, AL.add)

            for ls in range(NLS):
                l0 = lc * LC + ls * 128
                nc.sync.dma_start(
                    out=out[l0 : l0 + 128].rearrange("l d n -> l (d n)"),
                    in_=xts[ls],
                )

    nc.finalize()
    return nc


_NC_CACHE = None


def _get_nc():
    global _NC_CACHE
    if _NC_CACHE is None:
        _NC_CACHE = _build()
    return _NC_CACHE


def kernel(x: np.ndarray, W: np.ndarray, b: np.ndarray) -> np.ndarray:
    assert x.shape == (L, B, D, NB) and W.shape == (NB, D, 3 * D)
    from concourse.bass_utils import run_bass_kernel_spmd

    nc = _get_nc()
    x = np.asarray(x, dtype=np.float32)
    W = np.asarray(W, dtype=np.float32)
    b = np.asarray(b, dtype=np.float32)
    in_maps = [
        dict(x=np.ascontiguousarray(x[:, i]), w=W, bb=b) for i in range(N_CORES)
    ]
    results = run_bass_kernel_spmd(nc, in_maps, core_ids=list(range(N_CORES))).results
    return np.stack([results[i]["out"] for i in range(N_CORES)], axis=1)

